# revision 14
# baseline (speedup 1.0000x reference)
"""Trainium2 Bass kernel for nn_MultiHeadAttention_47330539602717 (V3).

Math (per batch b, head h; q/k projections are dead code in the reference):
    vpT    = Wv^T @ v_b^T + bv            (1024, 4096)   [d on partitions]
    A_h    = vp_h @ i_h^T                 (4096 s, 128 q) [s on partitions]
    expA   = exp(A - 28)                  softmax1 numerator (shift exact)
    S1[s]  = sum_q expA[s, q]             free-axis reduce
    attn1  = expA / S1                    per-(s,h) scalar multiply
    e2     = exp(9 * attn1)
    H[q',q]= sum_m e2[128m+q', q]         (torch raw .view fold)
    den2[q]= sum_{q'} H[q', q]
    w[q']  = sum_q H[q', q] / den2[q]
    x_h    = w @ i_h                      (64,)
    out_b  = concat_h(x_h) @ Wo + bo      (1, 1024)

Sharding: data-parallel over batch; core c handles batch b=c (16 heads).

V3 engine plan per core (vs V2: normalize moved DVE->GPSIMD, folds merged
to one in-place 2x add per m, per-m single-call reduce/exp2, drains split
ACT/DVE, small first chunks for pipeline fill):
    PE    : big GEMM (bf16, N<=512), A-matmuls (2-head block-diag, N=256),
            epilogue mms
    ACT   : exp1 per-g (PSUM src), exp2 per-m N=2048, drains for most db
    DVE   : S1 reduce per-m (1x), 1/S1, casts, fold add per-m (2x bf16),
            drains for some db, epilogue
    GPSIMD: attn1 = expA * RS1 broadcast multiply per-m
    DMA   : vT/Wv/Wo bf16 streams (~13 MB)
"""

import sys

import numpy as np

sys.path.insert(0, "/opt/trn_rl_repo")

from contextlib import ExitStack

import concourse.bacc as bacc
import concourse.tile as tile
from concourse import mybir
from concourse.bass_utils import run_bass_kernel_spmd

F32 = mybir.dt.float32
BF16 = mybir.dt.bfloat16
EXP = mybir.ActivationFunctionType.Exp
IDENT = mybir.ActivationFunctionType.Identity
AX = mybir.AxisListType.X
ADD = mybir.AluOpType.add
MULT = mybir.AluOpType.mult

B, LQ, S, D, H = 8, 128, 1024 * 4, 1024, 16
DK = D // H          # 64
KD = 8               # k blocks of 128 over D (contraction)
SHIFT = 28.0
SMOOTH = 9.0
# chunk sizes over s (sum = 4096); small first chunks fill the pipeline fast
CH = [256, 256] + [512] * 7
assert sum(CH) == S
# db indices whose PSUM->SBUF drain runs on DVE (rest on ACT)
DVE_DB = (5, 7)


def build_program():
    nc = bacc.Bacc("TRN2", target_bir_lowering=False, debug=False,
                   num_devices=8)

    vT_d = nc.dram_tensor("vT", [D, S], BF16, kind="ExternalInput").ap()
    Wv_d = nc.dram_tensor("Wv", [D, D], BF16, kind="ExternalInput").ap()
    Wo_d = nc.dram_tensor("Wo", [D, D], BF16, kind="ExternalInput").ap()
    bv_d = nc.dram_tensor("bv", [128, KD], F32, kind="ExternalInput").ap()
    iTbd_d = nc.dram_tensor("iTbd", [128, KD, 256], BF16,
                            kind="ExternalInput").ap()
    iN_d = nc.dram_tensor("iN", [128, H, DK], BF16, kind="ExternalInput").ap()
    bo_d = nc.dram_tensor("bo", [1, D], F32, kind="ExternalInput").ap()
    onesc_d = nc.dram_tensor("onesc", [128, 1], BF16, kind="ExternalInput").ap()
    onesr_d = nc.dram_tensor("onesr", [1, 128], BF16, kind="ExternalInput").ap()
    eye_d = nc.dram_tensor("eye", [128, 128], BF16, kind="ExternalInput").ap()
    out_d = nc.dram_tensor("out", [1, D], F32, kind="ExternalOutput").ap()

    with tile.TileContext(nc) as tc, ExitStack() as ctx:
        singles = ctx.enter_context(tc.tile_pool(name="singles", bufs=1))
        vtp = ctx.enter_context(tc.tile_pool(name="vtp", bufs=3))
        vpp = ctx.enter_context(tc.tile_pool(name="vpp", bufs=3))
        eap = ctx.enter_context(tc.tile_pool(name="eap", bufs=4))
        a9p = ctx.enter_context(tc.tile_pool(name="a9p", bufs=4))
        e2p = ctx.enter_context(tc.tile_pool(name="e2p", bufs=4))
        smp = ctx.enter_context(tc.tile_pool(name="smp", bufs=8))
        vp_ps = ctx.enter_context(
            tc.tile_pool(name="vp_ps", bufs=2, space="PSUM"))
        a_ps = ctx.enter_context(
            tc.tile_pool(name="a_ps", bufs=2, space="PSUM"))
        pf_ps = ctx.enter_context(
            tc.tile_pool(name="pf_ps", bufs=1, space="PSUM"))

        # ---- constants / weights ----
        # Wv split per k-chunk so chunk-0 GEMM k0 starts as soon as possible
        Wv_sb = singles.tile([128, KD, D], BF16)      # k-chunk k at [:, k, :]
        vT0_sb = vtp.tile([128, KD, 512], BF16, tag="vT")
        for k in range(KD):
            nc.sync.dma_start(out=Wv_sb[:, k, :],
                              in_=Wv_d[128 * k:128 * (k + 1), :])
            nc.sync.dma_start(out=vT0_sb[:, k, 0:CH[0]],
                              in_=vT_d[128 * k:128 * (k + 1), 0:CH[0]])
        iTbd_sb = singles.tile([128, KD, 256], BF16)  # block-diag i^T pairs
        nc.sync.dma_start(out=iTbd_sb, in_=iTbd_d)
        iN_sb = singles.tile([128, H, DK], BF16)      # natural i per head
        nc.sync.dma_start(out=iN_sb, in_=iN_d)
        bv_sb = singles.tile([128, KD], F32)          # bv[128*db+p] at [p, db]
        nc.sync.dma_start(out=bv_sb, in_=bv_d)
        bo_sb = singles.tile([1, D], F32)
        nc.sync.dma_start(out=bo_sb, in_=bo_d)
        onesc = singles.tile([128, 1], BF16)
        nc.sync.dma_start(out=onesc, in_=onesc_d)
        onesr = singles.tile([1, 128], BF16)
        nc.sync.dma_start(out=onesr, in_=onesr_d)
        Pfold = singles.tile([128, H, 128], BF16)     # head h at [:, h, :]
        nc.vector.memset(Pfold[:, 8:16, :], 0.0)
        eye_sb = singles.tile([128, 128], BF16)
        nc.sync.dma_start(out=eye_sb, in_=eye_d)
        PfoldPs = pf_ps.tile([128, 8, 128], F32)      # heads 0-7 fold on PE
        nshift = singles.tile([128, 1], F32)          # exp1 bias = -28
        nc.vector.memset(nshift, -SHIFT)
        zbias = singles.tile([128, 1], F32)           # exp2 bias = 0
        nc.vector.memset(zbias, 0.0)

        # ---- main loop (software-pipelined: chunk c+1's GEMM db-groups are
        # emitted between chunk c's m-iterations so each engine's FIFO
        # interleaves GEMM drains with softmax work) ----
        NCHK = len(CH)
        offs = [sum(CH[:c]) for c in range(NCHK)]
        vpT_tiles = {}

        def emit_gemm_group(c, db, vT_sb):
            nch = CH[c]
            vpT_sb = vpT_tiles[c]
            vp_p = vp_ps.tile([128, 512], F32, tag="vp")
            for k in range(KD):
                nc.tensor.matmul(
                    vp_p[:, 0:nch],
                    lhsT=Wv_sb[:, k, db * 128:(db + 1) * 128],
                    rhs=vT_sb[:, k, 0:nch],
                    start=(k == 0), stop=(k == KD - 1),
                )
            if db in DVE_DB:
                nc.vector.tensor_scalar(
                    out=vpT_sb[:, db, 0:nch], in0=vp_p[:, 0:nch],
                    scalar1=bv_sb[:, db:db + 1], scalar2=None, op0=ADD)
            else:
                nc.scalar.activation(
                    vpT_sb[:, db, 0:nch], vp_p[:, 0:nch],
                    IDENT, bias=bv_sb[:, db:db + 1])

        # deferred-stage softmax pipeline: stage1 (A mms, exp1, S1, 1/S1) at
        # iteration mm; stage2 (normalize-mult, exp2) for mm-1; fold for mm-2.
        # Keeps each engine's in-order FIFO free of waits on later stages.
        NM = S // 128            # 32 global m-tiles
        st = {}                  # global m -> dict of tiles

        def emit_stage1(mm, c, m):
            vpT_sb = vpT_tiles[c]
            expA = eap.tile([128, H, 128], BF16, tag="ea", name=f"ea{mm}")
            S1 = smp.tile([128, H], F32, tag="s1", name=f"s1_{mm}")
            RS1 = smp.tile([128, H], F32, tag="rs1", name=f"rs1_{mm}")
            for g in range(2):
                A_p = a_ps.tile([128, 4, 256], F32, tag="A", name=f"A{mm}_{g}")
                for jj in range(4):
                    j = 4 * g + jj
                    nc.tensor.matmul(
                        A_p[:, jj, :],
                        lhsT=vpT_sb[:, j, m * 128:(m + 1) * 128],
                        rhs=iTbd_sb[:, j, :],
                        start=True, stop=True,
                    )
                nc.scalar.activation(
                    expA[:, 8 * g:8 * g + 8, :].rearrange(
                        "p (j t) q -> p j (t q)", t=2),
                    A_p, EXP, bias=nshift)
            nc.vector.tensor_reduce(out=S1, in_=expA, axis=AX, op=ADD)
            nc.vector.reciprocal_approx_fast(RS1, S1)
            st[mm] = {"expA": expA, "RS1": RS1}

        def emit_stage2(mm):
            d = st[mm]
            attn1 = a9p.tile([128, H, 128], BF16, tag="a9", name=f"a9_{mm}")
            e2 = e2p.tile([128, H, 128], BF16, tag="e2", name=f"e2_{mm}")
            nc.gpsimd.tensor_mul(attn1, d["expA"],
                                 d["RS1"].broadcast_to([128, H, 128]))
            nc.scalar.activation(e2, attn1, EXP, bias=zbias, scale=SMOOTH)
            d["e2"] = e2

        def emit_fold(mm):
            e2 = st.pop(mm)["e2"]
            # heads 0-7 accumulate on PE (identity matmul into PSUM),
            # heads 8-15 in-place on DVE
            for t in range(2):
                nc.tensor.matmul(
                    PfoldPs.rearrange("p j q -> p (j q)")[
                        :, 512 * t:512 * (t + 1)],
                    lhsT=eye_sb,
                    rhs=e2.rearrange("p j q -> p (j q)")[
                        :, 512 * t:512 * (t + 1)],
                    start=(mm == 0), stop=(mm == NM - 1),
                    skip_group_check=True,
                )
            nc.vector.tensor_add(Pfold[:, 8:16, :], Pfold[:, 8:16, :],
                                 e2[:, 8:16, :])

        vT_tiles = {0: vT0_sb}

        def issue_vt_dma(c):
            if c >= NCHK or c in vT_tiles:
                return
            t = vtp.tile([128, KD, 512], BF16, tag="vT", name=f"vTc{c}")
            nc.sync.dma_start(
                out=t[:, :, 0:CH[c]],
                in_=vT_d[:, offs[c]:offs[c] + CH[c]].rearrange(
                    "(k p) s -> p k s", p=128))
            vT_tiles[c] = t

        issue_vt_dma(1)
        # prologue: chunk 0 GEMM is not overlapped with softmax work
        vpT_tiles[0] = vpp.tile([128, KD, 512], BF16, tag="vp", name="vpT0")
        for db in range(KD):
            emit_gemm_group(0, db, vT_tiles[0])

        mm = 0
        for c in range(NCHK):
            nsub = CH[c] // 128
            issue_vt_dma(c + 2)
            if c + 1 < NCHK:
                vpT_tiles[c + 1] = vpp.tile([128, KD, 512], BF16, tag="vp",
                                             name=f"vpT{c + 1}")
            per_m = KD // nsub      # next chunk's db-groups per m-iteration
            for m in range(nsub):
                emit_stage1(mm, c, m)
                if mm >= 1:
                    emit_stage2(mm - 1)
                if mm >= 2:
                    emit_fold(mm - 2)
                if c + 1 < NCHK:
                    for db in range(per_m * m, per_m * (m + 1)):
                        emit_gemm_group(c + 1, db, vT_tiles[c + 1])
                mm += 1
            vpT_tiles.pop(c, None)
            vT_tiles.pop(c)
        # flush deferred stages
        emit_stage2(NM - 1)
        emit_fold(NM - 2)
        emit_fold(NM - 1)

        # stage Wo in the freed vT slots for the epilogue
        Wo_t = []
        for n in range(2):
            Wo_h = vtp.tile([128, KD, 512], BF16, tag="vT")
            Wo_t.append(Wo_h)
            nc.sync.dma_start(
                out=Wo_h,
                in_=Wo_d[:, n * 512:(n + 1) * 512].rearrange(
                    "(k p) c -> p k c", p=128))

        # ---- epilogue ----
        # drain the PE-folded half (heads 0-7) from PSUM into Pfold
        for t in range(2):
            nc.scalar.activation(
                Pfold.rearrange("p j q -> p (j q)")[:, 512 * t:512 * (t + 1)],
                PfoldPs.rearrange("p j q -> p (j q)")[:, 512 * t:512 * (t + 1)],
                IDENT, bias=zbias)
        # den[q] = sum_q' Pfold[q', (h q)] via ones-matmuls (N=512 each)
        den_sb = singles.tile([1, H, 128], F32)
        for n in range(4):
            den_p = vp_ps.tile([1, 512], F32, tag="vp")
            nc.tensor.matmul(
                den_p,
                lhsT=onesc,
                rhs=Pfold[:, 4 * n:4 * n + 4, :].rearrange("p j q -> p (j q)"),
                start=True, stop=True, skip_group_check=True)
            nc.scalar.activation(
                den_sb[:, 4 * n:4 * n + 4, :].rearrange("p j q -> p (j q)"),
                den_p, IDENT, bias=zbias[0:1, :])
        rden = singles.tile([1, H, 128], F32)
        nc.vector.reciprocal_approx_fast(rden, den_sb)
        rdenb = singles.tile([1, H, 128], BF16)
        nc.vector.tensor_copy(rdenb, rden)

        # broadcast rden to 128 partitions via rank-1 matmuls, multiply by
        # Pfold and reduce over q -> w
        w_sb = singles.tile([128, H], F32)
        for n in range(2):
            R_p = a_ps.tile([128, 4, 256], F32, tag="A")
            for t in range(2):
                nc.tensor.matmul(
                    R_p.rearrange("p j q -> p (j q)")[:, 512 * t:512 * (t + 1)],
                    lhsT=onesr,
                    rhs=rdenb.rearrange("p h q -> p (h q)")[
                        :, 1024 * n + 512 * t:1024 * n + 512 * (t + 1)],
                    start=True, stop=True, skip_group_check=True)
            Hs = smp.tile([128, 8, 128], BF16, tag="hs")
            nc.vector.tensor_mul(Hs, Pfold[:, 8 * n:8 * n + 8, :],
                                 R_p.rearrange("p j (t q) -> p (j t) q", q=128))
            nc.vector.tensor_reduce(
                out=w_sb[:, 8 * n:8 * n + 8], in_=Hs, axis=AX, op=ADD)
        w_bf = singles.tile([128, H], BF16)
        nc.vector.tensor_copy(w_bf, w_sb)

        # x_h = i_h^T @ w_h  (64,) packed two heads per 128 partitions
        x_p = vp_ps.tile([128, KD], F32, tag="vp")
        for j in range(KD):
            for t in range(2):
                h = 2 * j + t
                nc.tensor.matmul(
                    x_p[64 * t:64 * t + 64, j:j + 1],
                    lhsT=iN_sb[:, h, :],
                    rhs=w_bf[:, h:h + 1],
                    start=True, stop=True, skip_group_check=True,
                )
        x_bf = singles.tile([128, KD], BF16)
        nc.vector.tensor_copy(x_bf, x_p)

        # out = x @ Wo + bo
        out_sb = singles.tile([1, D], F32)
        for n in range(2):
            o_p = a_ps.tile([1, 512], F32, tag="A")
            for j in range(KD):
                nc.tensor.matmul(
                    o_p,
                    lhsT=x_bf[:, j:j + 1],
                    rhs=Wo_t[n][:, j, :],
                    start=(j == 0), stop=(j == KD - 1),
                )
            nc.vector.tensor_add(out_sb[:, n * 512:(n + 1) * 512], o_p,
                                 bo_sb[:, n * 512:(n + 1) * 512])
        nc.sync.dma_start(out=out_d, in_=out_sb)

    nc.compile()
    return nc


def make_in_maps(v, i, Wv, bv, Wo, bo):
    """Shard + lay out inputs per core (core c = batch c)."""
    import ml_dtypes
    bf = ml_dtypes.bfloat16
    v = np.asarray(v, np.float32)
    i = np.asarray(i, np.float32)
    Wv_b = np.ascontiguousarray(np.asarray(Wv, np.float32)).astype(bf)
    Wo_b = np.ascontiguousarray(np.asarray(Wo, np.float32)).astype(bf)
    bv = np.asarray(bv, np.float32)
    bo = np.ascontiguousarray(np.asarray(bo, np.float32)).reshape(1, D)
    bv_sb = np.ascontiguousarray(bv.reshape(KD, 128).T)          # [p, db]
    onesc = np.ones((128, 1), np.float32).astype(bf)
    onesr = np.ones((1, 128), np.float32).astype(bf)
    eye = np.eye(128, dtype=np.float32).astype(bf)
    in_maps = []
    for b in range(B):
        hv = i[b * H:(b + 1) * H]                      # (16, 128, 64)
        iTbd = np.zeros((128, KD, 256), np.float32)
        for j in range(KD):
            iTbd[0:64, j, 0:128] = hv[2 * j].T         # head 2j
            iTbd[64:128, j, 128:256] = hv[2 * j + 1].T  # head 2j+1
        iN = np.ascontiguousarray(np.transpose(hv, (1, 0, 2)))  # (128,16,64)
        in_maps.append({
            "vT": np.ascontiguousarray(v[b].T).astype(bf),
            "Wv": Wv_b,
            "Wo": Wo_b,
            "bv": bv_sb,
            "iTbd": iTbd.astype(bf),
            "iN": iN,
            "bo": bo,
            "onesc": onesc,
            "onesr": onesr,
            "eye": eye,
        })
        in_maps[-1]["iN"] = in_maps[-1]["iN"].astype(bf)
    return in_maps


_NC_CACHE = None


def kernel(q, k, v, i, Wq, bq, Wk, bk, Wv, bv, Wo, bo):
    global _NC_CACHE
    if _NC_CACHE is None:
        _NC_CACHE = build_program()
    nc = _NC_CACHE
    in_maps = make_in_maps(v, i, Wv, bv, Wo, bo)
    res = run_bass_kernel_spmd(nc, in_maps, list(range(8)))
    rows = [res.results[c]["out"].reshape(1, D) for c in range(B)]
    return np.stack(rows, axis=0).astype(np.float32)  # (8, 1, 1024)


if __name__ == "__main__":
    build_program()
    print("compiled OK")
